# revision 1
# baseline (speedup 1.0000x reference)
"""Trainium2 Bass kernel for the note/wiki 3-way contraction + gate MLP.

Math (per note n):
    e[n]    = (wikivec * notevec[n]) @ W_emb.T + b_emb          # (C, K)
    attn[n] = sigmoid(e[n] @ W_att.T + b_att)                   # (C, K)
    s[n]    = sum_k attn[n]*e[n]*W_out[0,k] + b_out             # (C,)

Sharding: data-parallel over the 16 notes -> 2 notes per core on 8 cores.
wikivec / W_emb are replicated (pre-transposed, zero-padded to 10112 = 79*128
along the contraction axis, cast to bf16 on the host so the per-core HBM->SBUF
stream is ~10 MB and hides under the PE work).

Device layout (all v-major so the contraction dim sits on partitions):
  phase 1: for each of 79 v-tiles, scale wikivec^T[v,:] by notevec[n,v]
           (per-partition scalar; note0 on DVE, note1 on ACT) into one
           [128, 512] bf16 moving tile, then 2 matmuls (k-halves) accumulate
           e^T[k, (note,c)] into two PSUM banks over all 79 v-tiles.
  phase 2: bias via ACT Identity, bf16 copy, 4 matmuls for attn logits,
           sigmoid, gate, W_out contraction, + b_out, DMA out s [1, 512].
"""

import sys

if "/opt/trn_rl_repo" not in sys.path:
    sys.path.insert(0, "/opt/trn_rl_repo")

import numpy as np
import ml_dtypes

import concourse.bass as bass
import concourse.mybir as mybir
import concourse.tile as tile
from concourse import bacc
from concourse.bass_utils import run_bass_kernel_spmd

N_CORES = 8
N, C, V, K = 16, 256, 10000, 256
J = 79  # number of 128-row v-tiles (V padded to 10112)
BLK = 8  # v-tiles per DMA block (DMA-issue on the Sync queue is ~700ns/op)
J2 = 80  # J padded to a multiple of BLK (pad tile is all-zero)
NB = J2 // BLK
VP = J * 128
NLOC = N // N_CORES  # notes per core

F32 = mybir.dt.float32
BF16 = mybir.dt.bfloat16
BF16_NP = ml_dtypes.bfloat16

_NC_CACHE = {}


def _build_nc():
    nc = bacc.Bacc(None, target_bir_lowering=False)

    wikiT = nc.declare_dram_parameter("wikiT", [NB, 128, BLK * C], BF16, isOutput=False)
    wembT = nc.declare_dram_parameter("wembT", [NB, 128, BLK * K], BF16, isOutput=False)
    scales = nc.declare_dram_parameter("scales", [128, NLOC * J2], F32, isOutput=False)
    watT = nc.declare_dram_parameter("watT", [2, 128, K], BF16, isOutput=False)
    woutT = nc.declare_dram_parameter("woutT", [128, 2], F32, isOutput=False)
    bemb = nc.declare_dram_parameter("bemb", [128, 2], F32, isOutput=False)
    batt = nc.declare_dram_parameter("batt", [128, 2], F32, isOutput=False)
    bout = nc.declare_dram_parameter("bout", [1, 1], F32, isOutput=False)
    s_out = nc.declare_dram_parameter("s_out", [1, NLOC * C], F32, isOutput=True)

    NC2 = NLOC * C  # 512: (note, c) column block

    with tile.TileContext(nc) as tc:
        with (
            tc.tile_pool(name="const", bufs=1) as constp,
            tc.tile_pool(name="wt", bufs=4) as wtp,
            tc.tile_pool(name="et", bufs=4) as etp,
            tc.tile_pool(name="mov", bufs=4) as movp,
            tc.tile_pool(name="post", bufs=1) as postp,
            tc.tile_pool(name="psum", bufs=1, space="PSUM") as psp,
        ):
            sc = constp.tile([128, NLOC * J2], F32)
            nc.sync.dma_start(sc[:], scales[:])
            wat = constp.tile([128, 2 * K], BF16)
            nc.sync.dma_start(wat[:, 0:K], watT[0])
            nc.sync.dma_start(wat[:, K : 2 * K], watT[1])
            wout = constp.tile([128, 2], F32)
            nc.sync.dma_start(wout[:], woutT[:])
            be = constp.tile([128, 2], F32)
            nc.sync.dma_start(be[:], bemb[:])
            ba = constp.tile([128, 2], F32)
            nc.sync.dma_start(ba[:], batt[:])
            bo = constp.tile([1, 1], F32)
            nc.sync.dma_start(bo[:], bout[:])

            # Warmup reads: the activation engine only supports a single
            # sync-wait per instruction, so let ACT/DVE observe the constant
            # DMA semaphore lanes up front, one lane per tiny instruction.
            warm0 = constp.tile([128, 1], F32)
            nc.scalar.copy(warm0[:], be[:, 0:1])
            warm1 = constp.tile([128, 1], F32)
            nc.scalar.copy(warm1[:], ba[:, 0:1])
            warm2 = constp.tile([1, 1], F32)
            nc.scalar.copy(warm2[:], bo[:])
            warmd = constp.tile([128, 1], F32)
            nc.vector.tensor_copy(warmd[:], sc[:, 0:1])

            # e^T accumulators: [k-half 128, (note,c) 512] fp32, one bank each
            e_ps = [
                psp.tile([128, NC2], F32, name=f"e_ps{m}", tag=f"e_ps{m}")
                for m in range(2)
            ]

            for b in range(NB):
                wt = wtp.tile([128, BLK * C], BF16)
                nc.sync.dma_start(wt[:], wikiT[b])
                et = etp.tile([128, BLK * K], BF16)
                nc.sync.dma_start(et[:], wembT[b])
                for jj in range(BLK):
                    j = b * BLK + jj
                    wts = wt[:, jj * C : (jj + 1) * C]
                    mov = movp.tile([128, NC2], BF16)
                    # note0 on DVE, note1 on ACT (GpSimd shares SBUF ports
                    # with DVE and wrecks both when run concurrently)
                    nc.vector.tensor_scalar_mul(mov[:, 0:C], wts, sc[:, j : j + 1])
                    nc.scalar.mul(
                        mov[:, C : 2 * C], wts, mul=sc[:, J2 + j : J2 + j + 1]
                    )
                    st, sp = (j == 0), (j == J2 - 1)
                    for m in range(2):
                        nc.tensor.matmul(
                            e_ps[m][:],
                            et[:, jj * K + m * 128 : jj * K + (m + 1) * 128],
                            mov[:],
                            start=st,
                            stop=sp,
                        )

            # ---- phase 2: bias, attn logits, sigmoid, gate, W_out ----
            ef = []
            eb = []
            for m in range(2):
                ef_m = postp.tile([128, NC2], F32, tag=f"ef{m}")
                nc.scalar.activation(
                    ef_m[:],
                    e_ps[m][:],
                    mybir.ActivationFunctionType.Identity,
                    bias=be[:, m : m + 1],
                    scale=1.0,
                )
                eb_m = postp.tile([128, NC2], BF16, tag=f"eb{m}")
                nc.vector.tensor_copy(eb_m[:], ef_m[:])
                ef.append(ef_m)
                eb.append(eb_m)

            a_ps = [
                psp.tile([128, NC2], F32, name=f"a_ps{jm}", tag=f"a_ps{jm}")
                for jm in range(2)
            ]
            for kt in range(2):
                for jm in range(2):
                    nc.tensor.matmul(
                        a_ps[jm][:],
                        wat[:, kt * K + jm * 128 : kt * K + (jm + 1) * 128],
                        eb[kt][:],
                        start=(kt == 0),
                        stop=(kt == 1),
                    )

            v = []
            for jm in range(2):
                atn = postp.tile([128, NC2], F32, tag=f"atn{jm}")
                nc.scalar.activation(
                    atn[:],
                    a_ps[jm][:],
                    mybir.ActivationFunctionType.Sigmoid,
                    bias=ba[:, jm : jm + 1],
                    scale=1.0,
                )
                v_jm = postp.tile([128, NC2], F32, tag=f"v{jm}")
                nc.vector.tensor_mul(v_jm[:], atn[:], ef[jm][:])
                v.append(v_jm)

            s_ps = psp.tile([1, NC2], F32, tag="s_ps")
            for kt in range(2):
                nc.tensor.matmul(
                    s_ps[:],
                    wout[:, kt : kt + 1],
                    v[kt][:],
                    start=(kt == 0),
                    stop=(kt == 1),
                )
            s_sb = postp.tile([1, NC2], F32, tag="s_sb")
            nc.scalar.activation(
                s_sb[:],
                s_ps[:],
                mybir.ActivationFunctionType.Identity,
                bias=bo[0:1, 0:1],
                scale=1.0,
            )
            nc.sync.dma_start(s_out[:], s_sb[:])

    nc.compile()
    return nc


def _get_nc():
    if "nc" not in _NC_CACHE:
        _NC_CACHE["nc"] = _build_nc()
    return _NC_CACHE["nc"]


def _pad_T_tile(a):
    """(rows, V) -> zero-padded (NB, 128, BLK*rows) transposed block tiles,
    bf16; block b col jj*rows+c holds a.T[(b*BLK+jj)*128 + p, c]."""
    rows = a.shape[0]
    out = np.zeros((J2 * 128, rows), np.float32)
    out[:V] = a.T
    out = out.reshape(NB, BLK, 128, rows).transpose(0, 2, 1, 3)
    return np.ascontiguousarray(out.reshape(NB, 128, BLK * rows)).astype(BF16_NP)


def prep_inputs(notevec, wikivec, W_emb, b_emb, W_att, b_att, W_out, b_out):
    wikiT = _pad_T_tile(np.asarray(wikivec, np.float32))
    wembT = _pad_T_tile(np.asarray(W_emb, np.float32))
    watT = np.ascontiguousarray(
        np.asarray(W_att, np.float32).T.reshape(2, 128, K)
    ).astype(BF16_NP)
    woutT = np.ascontiguousarray(
        np.asarray(W_out, np.float32)[0].reshape(2, 128).T
    )
    bemb = np.ascontiguousarray(np.asarray(b_emb, np.float32).reshape(2, 128).T)
    batt = np.ascontiguousarray(np.asarray(b_att, np.float32).reshape(2, 128).T)
    bout = np.asarray(b_out, np.float32).reshape(1, 1)

    nv = np.zeros((N, J2 * 128), np.float32)
    nv[:, :V] = np.asarray(notevec, np.float32)
    in_maps = []
    for i in range(N_CORES):
        # scales[p, l*J2 + j] = notevec[2i+l, j*128+p]
        sc = np.ascontiguousarray(
            nv[i * NLOC : (i + 1) * NLOC].reshape(NLOC, J2, 128).transpose(2, 0, 1)
        ).reshape(128, NLOC * J2)
        in_maps.append(
            {
                "wikiT": wikiT,
                "wembT": wembT,
                "scales": np.ascontiguousarray(sc),
                "watT": watT,
                "woutT": woutT,
                "bemb": bemb,
                "batt": batt,
                "bout": bout,
            }
        )
    return in_maps


def run(in_maps, **kw):
    nc = _get_nc()
    return run_bass_kernel_spmd(nc, in_maps, list(range(N_CORES)), **kw)


def kernel(notevec, wikivec, W_emb, b_emb, W_att, b_att, W_out, b_out):
    in_maps = prep_inputs(
        notevec, wikivec, W_emb, b_emb, W_att, b_att, W_out, b_out
    )
    res = run(in_maps)
    out = np.concatenate(
        [r["s_out"].reshape(NLOC, C) for r in res.results], axis=0
    )
    return out.astype(np.float32)



# revision 7
# speedup vs baseline: 1.0237x; 1.0237x over previous
"""Trainium2 Bass kernel for the note/wiki 3-way contraction + gate MLP.

Math (per note n):
    e[n]    = (wikivec * notevec[n]) @ W_emb.T + b_emb          # (C, K)
    attn[n] = sigmoid(e[n] @ W_att.T + b_att)                   # (C, K)
    s[n]    = sum_k attn[n]*e[n]*W_out[0,k] + b_out             # (C,)

Sharding: data-parallel over the 16 notes -> 2 notes per core on 8 cores.
wikivec / W_emb are replicated, pre-transposed to v-major bf16 on the host
and interleaved per 128-row v-tile as [wiki 256 | wemb 256] so each block
needs a single DMA.

Device phase 1 (per 128-row v-tile): scale the wiki columns by notevec
(DVE cols 0..SPLIT, ACT cols SPLIT..512 of the (note, c) block), then two
matmuls (k-halves) accumulate e^T[k, (note,c)] into two PSUM banks across
all 79 v-tiles.  Dummy matmuls on a zero tile ramp the PE p-state during
the DMA prologue, and the first block streams in 2-tile chunks so real
matmuls start as early as possible.

Phase 2: bias via ACT Identity, bf16 copy, 4 matmuls for attn logits,
sigmoid, gate, W_out contraction, + b_out, DMA out s [1, 512].
"""

import sys

if "/opt/trn_rl_repo" not in sys.path:
    sys.path.insert(0, "/opt/trn_rl_repo")

import numpy as np
import ml_dtypes

import concourse.bass as bass
import concourse.mybir as mybir
import concourse.tile as tile
from concourse import bacc
from concourse.bass_utils import run_bass_kernel_spmd

N_CORES = 8
N, C, V, K = 16, 256, 10000, 256
J = 79  # number of 128-row v-tiles (V=10000 -> 78 full + 1 partial)
J2 = 80  # scales stride per note
NLOC = N // N_CORES  # notes per core
NC2 = NLOC * C  # 512: (note, c) column block
TW = C + K  # 512: combined [wiki | wemb] tile width
HEAD = 4  # first block split into HEAD chunks of HCH tiles
HCH = 2
NBLK = 9  # full blocks after the chunked head block
BLK = 8  # v-tiles per block
SPLIT = 320  # mov columns 0..SPLIT on DVE, SPLIT..512 on ACT
NDUMMY = 8  # PE p-state warmup matmuls

F32 = mybir.dt.float32
BF16 = mybir.dt.bfloat16
BF16_NP = ml_dtypes.bfloat16

PACK_W = 167  # 160 scales | 2 wout | 2 bemb | 2 batt | 1 bout

_NC_CACHE = {}


def _build_nc():
    nc = bacc.Bacc(None, target_bir_lowering=False)

    blk_d = nc.declare_dram_parameter("blk", [128, J * TW], BF16, isOutput=False)
    pack_d = nc.declare_dram_parameter("pack", [128, PACK_W], F32, isOutput=False)
    watT = nc.declare_dram_parameter("watT", [128, 2 * K], BF16, isOutput=False)
    s_out = nc.declare_dram_parameter("s_out", [1, NC2], F32, isOutput=True)

    with tile.TileContext(nc) as tc:
        with (
            tc.tile_pool(name="const", bufs=1) as constp,
            tc.tile_pool(name="blk", bufs=3) as blkp,
            tc.tile_pool(name="mov", bufs=4) as movp,
            tc.tile_pool(name="post", bufs=1) as postp,
            tc.tile_pool(name="psum", bufs=1, space="PSUM") as psp,
        ):
            pk = constp.tile([128, PACK_W], F32)
            nc.sync.dma_start(pk[:], pack_d[:])
            sc = pk[:, 0 : NLOC * J2]
            wout = pk[:, 160:162]
            be = pk[:, 162:164]
            ba = pk[:, 164:166]
            bo = pk[0:1, 166:167]

            # PE p-state warmup: matmuls on a zeroed tile keep the PE busy
            # during the DMA prologue so the clock is at full speed when the
            # real stream starts.
            scr = constp.tile([128, NC2], BF16)
            nc.vector.memset(scr[:], 0.0)
            scr_ps = psp.tile([128, NC2], F32, name="scr_ps", tag="scr_ps")
            for _ in range(NDUMMY):
                nc.tensor.matmul(
                    scr_ps[:], scr[:, 0:128], scr[:], start=True, stop=True
                )

            # Let ACT/DVE observe the pack-DMA semaphore lane up front (the
            # activation engine only supports a single sync-wait per
            # instruction; their first loop op also waits on a block DMA).
            warm0 = constp.tile([128, 1], F32)
            nc.scalar.copy(warm0[:], pk[:, 162:163])
            warmd = constp.tile([128, 1], F32)
            nc.vector.tensor_copy(warmd[:], pk[:, 0:1])

            # block schedule: (n_tiles, dram_row_offset), head chunks first
            sched = [(HCH, i * HCH) for i in range(HEAD)]
            base = HEAD * HCH
            for b in range(NBLK):
                sched.append((BLK, base + b * BLK))

            wat = constp.tile([128, 2 * K], BF16)

            # e^T accumulators: [k-half 128, (note,c) 512] fp32, one bank each
            e_ps = [
                psp.tile([128, NC2], F32, name=f"e_ps{m}", tag=f"e_ps{m}")
                for m in range(2)
            ]

            for bi, (ntiles, row0) in enumerate(sched):
                bt = blkp.tile([128, BLK * TW], BF16)
                nval = min(ntiles, J - row0)
                nc.sync.dma_start(
                    bt[:, 0 : nval * TW],
                    blk_d[:, row0 * TW : (row0 + nval) * TW],
                )
                if bi == 2:
                    # W_att is only needed in phase 2; issue after the
                    # stream is primed.
                    nc.sync.dma_start(wat[:], watT[:])
                for jj in range(nval):
                    g = row0 + jj
                    wts = bt[:, jj * TW : jj * TW + C]
                    emb = bt[:, jj * TW + C : (jj + 1) * TW]
                    mov = movp.tile([128, NC2], BF16)
                    # note0 fully on DVE, note1 split DVE/ACT so no single
                    # engine is above the PE's 426ns/tile budget
                    nc.vector.tensor_scalar_mul(
                        mov[:, 0:C], wts, sc[:, g : g + 1]
                    )
                    nc.vector.tensor_scalar_mul(
                        mov[:, C:SPLIT],
                        wts[:, 0 : SPLIT - C],
                        sc[:, J2 + g : J2 + g + 1],
                    )
                    nc.scalar.mul(
                        mov[:, SPLIT:NC2],
                        wts[:, SPLIT - C : C],
                        mul=sc[:, J2 + g : J2 + g + 1],
                    )
                    st, sp = (g == 0), (g == J - 1)
                    for m in range(2):
                        nc.tensor.matmul(
                            e_ps[m][:],
                            emb[:, m * 128 : (m + 1) * 128],
                            mov[:],
                            start=st,
                            stop=sp,
                        )

            # ---- phase 2: bias, attn logits, sigmoid, gate, W_out ----
            ef = []
            eb = []
            for m in range(2):
                ef_m = postp.tile([128, NC2], F32, tag=f"ef{m}")
                nc.scalar.activation(
                    ef_m[:],
                    e_ps[m][:],
                    mybir.ActivationFunctionType.Identity,
                    bias=be[:, m : m + 1],
                    scale=1.0,
                )
                eb_m = postp.tile([128, NC2], BF16, tag=f"eb{m}")
                nc.vector.tensor_copy(eb_m[:], ef_m[:])
                ef.append(ef_m)
                eb.append(eb_m)

            a_ps = [
                psp.tile([128, NC2], F32, name=f"a_ps{jm}", tag=f"a_ps{jm}")
                for jm in range(2)
            ]
            for kt in range(2):
                for jm in range(2):
                    nc.tensor.matmul(
                        a_ps[jm][:],
                        wat[:, kt * K + jm * 128 : kt * K + (jm + 1) * 128],
                        eb[kt][:],
                        start=(kt == 0),
                        stop=(kt == 1),
                    )

            v = []
            for jm in range(2):
                atn = postp.tile([128, NC2], F32, tag=f"atn{jm}")
                nc.scalar.activation(
                    atn[:],
                    a_ps[jm][:],
                    mybir.ActivationFunctionType.Sigmoid,
                    bias=ba[:, jm : jm + 1],
                    scale=1.0,
                )
                v_jm = postp.tile([128, NC2], F32, tag=f"v{jm}")
                nc.vector.tensor_mul(v_jm[:], atn[:], ef[jm][:])
                v.append(v_jm)

            s_ps = psp.tile([1, NC2], F32, tag="s_ps")
            for kt in range(2):
                nc.tensor.matmul(
                    s_ps[:],
                    wout[:, kt : kt + 1],
                    v[kt][:],
                    start=(kt == 0),
                    stop=(kt == 1),
                )
            s_sb = postp.tile([1, NC2], F32, tag="s_sb")
            nc.scalar.activation(
                s_sb[:],
                s_ps[:],
                mybir.ActivationFunctionType.Identity,
                bias=bo[:],
                scale=1.0,
            )
            nc.sync.dma_start(s_out[:], s_sb[:])

    nc.compile()
    return nc


def _get_nc():
    if "nc" not in _NC_CACHE:
        _NC_CACHE["nc"] = _build_nc()
    return _NC_CACHE["nc"]


def prep_inputs(notevec, wikivec, W_emb, b_emb, W_att, b_att, W_out, b_out):
    # blk[g] = [128, 512] bf16: cols 0:256 wikiT rows g*128..g*128+128,
    # cols 256:512 wembT same rows (zero-padded past V)
    both = np.zeros((J * 128, TW), np.float32)
    both[:V, 0:C] = np.asarray(wikivec, np.float32).T
    both[:V, C:TW] = np.asarray(W_emb, np.float32).T
    # partition-major: blk[p, g*TW + c] = tile g, sbuf partition p, col c
    blk = np.ascontiguousarray(
        both.reshape(J, 128, TW).transpose(1, 0, 2).reshape(128, J * TW)
    ).astype(BF16_NP)

    # watT[p, kt*K + j] = W_att[j, kt*128+p]
    watT = np.zeros((128, 2 * K), np.float32)
    wa = np.asarray(W_att, np.float32)
    for kt in range(2):
        watT[:, kt * K : (kt + 1) * K] = wa[:, kt * 128 : (kt + 1) * 128].T
    watT = np.ascontiguousarray(watT).astype(BF16_NP)

    nv = np.zeros((N, J2 * 128), np.float32)
    nv[:, :V] = np.asarray(notevec, np.float32)

    pack_common = np.zeros((128, PACK_W), np.float32)
    pack_common[:, 160:162] = np.asarray(W_out, np.float32)[0].reshape(2, 128).T
    pack_common[:, 162:164] = np.asarray(b_emb, np.float32).reshape(2, 128).T
    pack_common[:, 164:166] = np.asarray(b_att, np.float32).reshape(2, 128).T
    pack_common[:, 166] = np.asarray(b_out, np.float32)[0]

    in_maps = []
    for i in range(N_CORES):
        pack = pack_common.copy()
        # scales[p, l*J2 + g] = notevec[2i+l, g*128+p]
        pack[:, 0 : NLOC * J2] = (
            nv[i * NLOC : (i + 1) * NLOC]
            .reshape(NLOC, J2, 128)
            .transpose(2, 0, 1)
            .reshape(128, NLOC * J2)
        )
        in_maps.append(
            {
                "blk": blk,
                "pack": np.ascontiguousarray(pack),
                "watT": watT,
            }
        )
    return in_maps


def run(in_maps, **kw):
    nc = _get_nc()
    return run_bass_kernel_spmd(nc, in_maps, list(range(N_CORES)), **kw)


def kernel(notevec, wikivec, W_emb, b_emb, W_att, b_att, W_out, b_out):
    in_maps = prep_inputs(
        notevec, wikivec, W_emb, b_emb, W_att, b_att, W_out, b_out
    )
    res = run(in_maps)
    out = np.concatenate(
        [r["s_out"].reshape(NLOC, C) for r in res.results], axis=0
    )
    return out.astype(np.float32)


# revision 16
# speedup vs baseline: 1.0660x; 1.0413x over previous
"""Trainium2 Bass kernel for the note/wiki 3-way contraction + gate MLP.

Math (per note n):
    e[n]    = (wikivec * notevec[n]) @ W_emb.T + b_emb          # (C, K)
    attn[n] = sigmoid(e[n] @ W_att.T + b_att)                   # (C, K)
    s[n]    = sum_k attn[n]*e[n]*W_out[0,k] + b_out             # (C,)

Sharding: data-parallel over the 16 notes -> 2 notes per core on 8 cores.
wikivec / W_emb are replicated, pre-transposed to v-major bf16 on the host
and interleaved per 128-row v-tile as [wiki 256 | wemb 256] so each block
needs a single DMA.

Device phase 1 (per 128-row v-tile): scale the wiki columns by notevec
(DVE cols 0..SPLIT, ACT cols SPLIT..512 of the (note, c) block), then two
matmuls (k-halves) accumulate e^T[k, (note,c)] into two PSUM banks across
all 79 v-tiles.  Dummy matmuls on a zero tile ramp the PE p-state during
the DMA prologue, and the first block streams in 2-tile chunks so real
matmuls start as early as possible.

Phase 2: bias via ACT Identity, bf16 copy, 4 matmuls for attn logits,
sigmoid, gate, W_out contraction, + b_out, DMA out s [1, 512].
"""

import sys

if "/opt/trn_rl_repo" not in sys.path:
    sys.path.insert(0, "/opt/trn_rl_repo")

import numpy as np
import ml_dtypes

import concourse.bass as bass
import concourse.mybir as mybir
import concourse.tile as tile
from concourse import bacc
from concourse.bass_utils import run_bass_kernel_spmd

N_CORES = 8
N, C, V, K = 16, 256, 10000, 256
J = 79  # number of 128-row v-tiles (V=10000 -> 78 full + 1 partial)
J2 = 80  # scales stride per note
NLOC = N // N_CORES  # notes per core
NC2 = NLOC * C  # 512: (note, c) column block
TW = C + K  # 512: combined [wiki | wemb] tile width
HEAD = 4  # first block split into HEAD chunks of HCH tiles
HCH = 2
NBLK = 9  # full blocks after the chunked head block
BLK = 8  # v-tiles per block
NDUMMY = 3  # PE p-state warmup matmuls
# whole-tile producer schedule: each v-tile's two scaling ops run on ONE
# engine; DVE (~572ns/tile) takes 5 of 8, ACT (~980ns/tile) takes 3 of 8,
# so the aggregate rate beats the PE's 426ns/tile and neither engine
# sits on the critical path.
ACT_TILES = {2, 4, 7}

F32 = mybir.dt.float32
BF16 = mybir.dt.bfloat16
BF16_NP = ml_dtypes.bfloat16

PACK_W = 167  # 160 scales | 2 wout | 2 bemb | 2 batt | 1 bout

_NC_CACHE = {}


def _build_nc():
    nc = bacc.Bacc(None, target_bir_lowering=False)

    blk_d = nc.declare_dram_parameter("blk", [128, J * TW], BF16, isOutput=False)
    pack_d = nc.declare_dram_parameter("pack", [128, PACK_W], F32, isOutput=False)
    watT = nc.declare_dram_parameter("watT", [128, 2 * K + 2], BF16, isOutput=False)
    s_out = nc.declare_dram_parameter("s_out", [1, NC2], F32, isOutput=True)

    with tile.TileContext(nc) as tc:
        with (
            tc.tile_pool(name="const", bufs=1) as constp,
            tc.tile_pool(name="blk", bufs=3) as blkp,
            tc.tile_pool(name="mov", bufs=4) as movp,
            tc.tile_pool(name="post", bufs=1) as postp,
            tc.tile_pool(name="psum", bufs=1, space="PSUM") as psp,
        ):
            pk = constp.tile([128, PACK_W], F32)
            nc.sync.dma_start(pk[:], pack_d[:])
            sc = pk[:, 0 : NLOC * J2]
            be = pk[:, 162:164]
            ba = pk[:, 164:166]
            bo = pk[0:1, 166:167]

            # PE p-state warmup: matmuls on a zeroed tile keep the PE busy
            # during the DMA prologue so the clock is at full speed when the
            # real stream starts.
            scr = constp.tile([128, NC2], BF16)
            nc.vector.memset(scr[:], 0.0)
            scr_ps = psp.tile([128, NC2], F32, name="scr_ps", tag="scr_ps")
            for _ in range(NDUMMY):
                nc.tensor.matmul(
                    scr_ps[:], scr[:, 0:128], scr[:], start=True, stop=True
                )

            # Let ACT/DVE observe the pack-DMA semaphore lane up front (the
            # activation engine only supports a single sync-wait per
            # instruction; their first loop op also waits on a block DMA).
            warm0 = constp.tile([128, 1], F32)
            nc.scalar.copy(warm0[:], pk[:, 162:163])
            warmd = constp.tile([128, 1], F32)
            nc.vector.tensor_copy(warmd[:], pk[:, 0:1])

            # block schedule: (n_tiles, dram_row_offset), head chunks first
            sched = [(HCH, i * HCH) for i in range(HEAD)]
            base = HEAD * HCH
            for b in range(NBLK):
                sched.append((BLK, base + b * BLK))

            wat = constp.tile([128, 2 * K + 2], BF16)
            wout = wat[:, 2 * K : 2 * K + 2]

            # e^T accumulators: [k-half 128, (note,c) 512] fp32, one bank each
            e_ps = [
                psp.tile([128, NC2], F32, name=f"e_ps{m}", tag=f"e_ps{m}")
                for m in range(2)
            ]

            for bi, (ntiles, row0) in enumerate(sched):
                bt = blkp.tile([128, BLK * TW], BF16)
                nval = min(ntiles, J - row0)
                nc.sync.dma_start(
                    bt[:, 0 : nval * TW],
                    blk_d[:, row0 * TW : (row0 + nval) * TW],
                )
                if bi == 6:
                    # W_att is only needed in phase 2; issue late so it
                    # doesn't delay the block stream.
                    nc.sync.dma_start(wat[:], watT[:])
                for jj in range(nval):
                    g = row0 + jj
                    wts = bt[:, jj * TW : jj * TW + C]
                    emb = bt[:, jj * TW + C : (jj + 1) * TW]
                    mov = movp.tile([128, NC2], BF16)
                    if g % 8 in ACT_TILES:
                        nc.scalar.mul(mov[:, 0:C], wts, mul=sc[:, g : g + 1])
                        nc.scalar.mul(
                            mov[:, C:NC2], wts, mul=sc[:, J2 + g : J2 + g + 1]
                        )
                    else:
                        nc.vector.tensor_scalar_mul(
                            mov[:, 0:C], wts, sc[:, g : g + 1]
                        )
                        nc.vector.tensor_scalar_mul(
                            mov[:, C:NC2], wts, sc[:, J2 + g : J2 + g + 1]
                        )
                    st, sp = (g == 0), (g == J - 1)
                    for m in range(2):
                        nc.tensor.matmul(
                            e_ps[m][:],
                            emb[:, m * 128 : (m + 1) * 128],
                            mov[:],
                            start=st,
                            stop=sp,
                        )

            # ---- phase 2: bias, attn logits, sigmoid, gate, W_out ----
            eb = []
            for m in range(2):
                eb_m = postp.tile([128, NC2], BF16, tag=f"eb{m}")
                nc.scalar.activation(
                    eb_m[:],
                    e_ps[m][:],
                    mybir.ActivationFunctionType.Identity,
                    bias=be[:, m : m + 1],
                    scale=1.0,
                )
                eb.append(eb_m)

            a_ps = [
                psp.tile([128, NC2], F32, name=f"a_ps{jm}", tag=f"a_ps{jm}")
                for jm in range(2)
            ]
            for kt in range(2):
                for jm in range(2):
                    nc.tensor.matmul(
                        a_ps[jm][:],
                        wat[:, kt * K + jm * 128 : kt * K + (jm + 1) * 128],
                        eb[kt][:],
                        start=(kt == 0),
                        stop=(kt == 1),
                    )

            v = []
            for jm in range(2):
                atn = postp.tile([128, NC2], F32, tag=f"atn{jm}")
                nc.scalar.activation(
                    atn[:],
                    a_ps[jm][:],
                    mybir.ActivationFunctionType.Sigmoid,
                    bias=ba[:, jm : jm + 1],
                    scale=1.0,
                )
                v_jm = postp.tile([128, NC2], BF16, tag=f"v{jm}")
                nc.vector.tensor_mul(v_jm[:], atn[:], eb[jm][:])
                v.append(v_jm)

            s_ps = psp.tile([1, NC2], F32, tag="s_ps")
            for kt in range(2):
                nc.tensor.matmul(
                    s_ps[:],
                    wout[:, kt : kt + 1],
                    v[kt][:],
                    start=(kt == 0),
                    stop=(kt == 1),
                )
            s_sb = postp.tile([1, NC2], F32, tag="s_sb")
            nc.scalar.activation(
                s_sb[:],
                s_ps[:],
                mybir.ActivationFunctionType.Identity,
                bias=bo[:],
                scale=1.0,
            )
            nc.sync.dma_start(s_out[:], s_sb[:])

    nc.compile()
    return nc


def _get_nc():
    if "nc" not in _NC_CACHE:
        _NC_CACHE["nc"] = _build_nc()
    return _NC_CACHE["nc"]


def prep_inputs(notevec, wikivec, W_emb, b_emb, W_att, b_att, W_out, b_out):
    # blk[g] = [128, 512] bf16: cols 0:256 wikiT rows g*128..g*128+128,
    # cols 256:512 wembT same rows (zero-padded past V)
    both = np.zeros((J * 128, TW), np.float32)
    both[:V, 0:C] = np.asarray(wikivec, np.float32).T
    both[:V, C:TW] = np.asarray(W_emb, np.float32).T
    # partition-major: blk[p, g*TW + c] = tile g, sbuf partition p, col c
    blk = np.ascontiguousarray(
        both.reshape(J, 128, TW).transpose(1, 0, 2).reshape(128, J * TW)
    ).astype(BF16_NP)

    # watT[p, kt*K + j] = W_att[j, kt*128+p]; cols 512:514 hold W_out
    watT = np.zeros((128, 2 * K + 2), np.float32)
    wa = np.asarray(W_att, np.float32)
    for kt in range(2):
        watT[:, kt * K : (kt + 1) * K] = wa[:, kt * 128 : (kt + 1) * 128].T
    watT[:, 2 * K : 2 * K + 2] = np.asarray(W_out, np.float32)[0].reshape(2, 128).T
    watT = np.ascontiguousarray(watT).astype(BF16_NP)

    nv = np.zeros((N, J2 * 128), np.float32)
    nv[:, :V] = np.asarray(notevec, np.float32)

    pack_common = np.zeros((128, PACK_W), np.float32)
    pack_common[:, 162:164] = np.asarray(b_emb, np.float32).reshape(2, 128).T
    pack_common[:, 164:166] = np.asarray(b_att, np.float32).reshape(2, 128).T
    pack_common[:, 166] = np.asarray(b_out, np.float32)[0]

    in_maps = []
    for i in range(N_CORES):
        pack = pack_common.copy()
        # scales[p, l*J2 + g] = notevec[2i+l, g*128+p]
        pack[:, 0 : NLOC * J2] = (
            nv[i * NLOC : (i + 1) * NLOC]
            .reshape(NLOC, J2, 128)
            .transpose(2, 0, 1)
            .reshape(128, NLOC * J2)
        )
        in_maps.append(
            {
                "blk": blk,
                "pack": np.ascontiguousarray(pack),
                "watT": watT,
            }
        )
    return in_maps


def run(in_maps, **kw):
    nc = _get_nc()
    return run_bass_kernel_spmd(nc, in_maps, list(range(N_CORES)), **kw)


def kernel(notevec, wikivec, W_emb, b_emb, W_att, b_att, W_out, b_out):
    in_maps = prep_inputs(
        notevec, wikivec, W_emb, b_emb, W_att, b_att, W_out, b_out
    )
    res = run(in_maps)
    out = np.concatenate(
        [r["s_out"].reshape(NLOC, C) for r in res.results], axis=0
    )
    return out.astype(np.float32)


# revision 24
# speedup vs baseline: 1.1357x; 1.0654x over previous
"""Trainium2 Bass kernel for the note/wiki 3-way contraction + gate MLP.

Math (per note n):
    e[n]    = (wikivec * notevec[n]) @ W_emb.T + b_emb          # (C, K)
    attn[n] = sigmoid(e[n] @ W_att.T + b_att)                   # (C, K)
    s[n]    = sum_k attn[n]*e[n]*W_out[0,k] + b_out             # (C,)

Sharding: data-parallel over the 16 notes -> 2 notes per core on 8 cores.
wikivec / W_emb are replicated, pre-transposed to v-major bf16 on the host
and interleaved per 128-row v-tile as [wiki 256 | wemb 256] so each block
needs a single DMA.

Device phase 1 (per 128-row v-tile): scale the wiki columns by notevec
(DVE cols 0..SPLIT, ACT cols SPLIT..512 of the (note, c) block), then two
matmuls (k-halves) accumulate e^T[k, (note,c)] into two PSUM banks across
all 79 v-tiles.  Dummy matmuls on a zero tile ramp the PE p-state during
the DMA prologue, and the first block streams in 2-tile chunks so real
matmuls start as early as possible.

Phase 2: bias via ACT Identity, bf16 copy, 4 matmuls for attn logits,
sigmoid, gate, W_out contraction, + b_out, DMA out s [1, 512].
"""

import sys

if "/opt/trn_rl_repo" not in sys.path:
    sys.path.insert(0, "/opt/trn_rl_repo")

import numpy as np
import ml_dtypes

import concourse.bass as bass
import concourse.mybir as mybir
import concourse.tile as tile
from concourse import bacc
from concourse.bass_utils import run_bass_kernel_spmd

N_CORES = 8
N, C, V, K = 16, 256, 10000, 256
J = 79  # number of 128-row v-tiles (V=10000 -> 78 full + 1 partial)
J2 = 80  # scales stride per note
NLOC = N // N_CORES  # notes per core
NC2 = NLOC * C  # 512: (note, c) column block
TW = C + K  # 512: combined [wiki | wemb] tile width
BLK = 4  # v-tiles per DMA chunk (small chunks -> smooth arrival pipeline)
NDUMMY = 2  # PE p-state warmup matmuls
# whole-tile producer schedule: each v-tile's two scaling ops run on ONE
# engine; DVE (~572ns/tile) takes 11 of 16, ACT (~980ns/tile) takes 5 of
# 16 at an even spacing of 3, so the aggregate rate beats the PE's
# 426ns/tile and neither engine sits on the critical path.
ACT_TILES = {1, 4, 7, 10, 13}
# chunks issued on the ACT HWDGE queue (rest on Sync) so neither queue's
# descriptor serialization starves the stream
ACT_CHUNKS = {3, 6, 9, 12, 15, 18}

F32 = mybir.dt.float32
BF16 = mybir.dt.bfloat16
BF16_NP = ml_dtypes.bfloat16

PACK_W = 167  # 160 scales | 2 wout | 2 bemb | 2 batt | 1 bout

_NC_CACHE = {}


def _build_nc():
    nc = bacc.Bacc(None, target_bir_lowering=False)

    blk_d = nc.declare_dram_parameter("blk", [128, J * TW], BF16, isOutput=False)
    pack_d = nc.declare_dram_parameter("pack", [128, PACK_W], F32, isOutput=False)
    watT = nc.declare_dram_parameter("watT", [128, 2 * K + 2], BF16, isOutput=False)
    s_out = nc.declare_dram_parameter("s_out", [1, NC2], F32, isOutput=True)

    with tile.TileContext(nc) as tc:
        with (
            tc.tile_pool(name="const", bufs=1) as constp,
            tc.tile_pool(name="blk", bufs=6) as blkp,
            tc.tile_pool(name="mov", bufs=6) as movp,
            tc.tile_pool(name="post", bufs=1) as postp,
            tc.tile_pool(name="psum", bufs=1, space="PSUM") as psp,
        ):
            pk = constp.tile([128, PACK_W], F32)
            nc.sync.dma_start(pk[:], pack_d[:])
            sc = pk[:, 0 : NLOC * J2]
            be = pk[:, 162:164]
            ba = pk[:, 164:166]
            bo = pk[0:1, 166:167]

            # PE p-state warmup: matmuls on a zeroed tile keep the PE busy
            # during the DMA prologue so the clock is at full speed when the
            # real stream starts.  GpSimd memset: it starts earliest.
            scr = constp.tile([128, NC2], BF16)
            nc.gpsimd.memset(scr[:], 0.0)
            scr_ps = psp.tile([128, NC2], F32, name="scr_ps", tag="scr_ps")
            for _ in range(NDUMMY):
                nc.tensor.matmul(
                    scr_ps[:], scr[:, 0:128], scr[:], start=True, stop=True
                )

            # Let ACT/DVE observe the pack-DMA semaphore lane up front (the
            # activation engine only supports a single sync-wait per
            # instruction; their first loop op also waits on a block DMA).
            warm0 = constp.tile([128, 1], F32)
            nc.scalar.copy(warm0[:], pk[:, 162:163])
            warmd = constp.tile([128, 1], F32)
            nc.vector.tensor_copy(warmd[:], pk[:, 0:1])

            wat = constp.tile([128, 2 * K + 2], BF16)
            wout = wat[:, 2 * K : 2 * K + 2]

            # e^T accumulators: [k-half 128, (note,c) 512] fp32, one bank each
            e_ps = [
                psp.tile([128, NC2], F32, name=f"e_ps{m}", tag=f"e_ps{m}")
                for m in range(2)
            ]

            nchunks = (J + BLK - 1) // BLK
            for bi in range(nchunks):
                row0 = bi * BLK
                bt = blkp.tile([128, BLK * TW], BF16)
                nval = min(BLK, J - row0)
                q = nc.scalar if bi in ACT_CHUNKS else nc.sync
                q.dma_start(
                    bt[:, 0 : nval * TW],
                    blk_d[:, row0 * TW : (row0 + nval) * TW],
                )
                if bi == 12:
                    # W_att is only needed in phase 2; issue late so it
                    # doesn't delay the block stream.
                    nc.sync.dma_start(wat[:], watT[:])
                for jj in range(nval):
                    g = row0 + jj
                    wts = bt[:, jj * TW : jj * TW + C]
                    emb = bt[:, jj * TW + C : (jj + 1) * TW]
                    mov = movp.tile([128, NC2], BF16)
                    if g % 16 in ACT_TILES:
                        nc.scalar.mul(mov[:, 0:C], wts, mul=sc[:, g : g + 1])
                        nc.scalar.mul(
                            mov[:, C:NC2], wts, mul=sc[:, J2 + g : J2 + g + 1]
                        )
                    else:
                        nc.vector.tensor_scalar_mul(
                            mov[:, 0:C], wts, sc[:, g : g + 1]
                        )
                        nc.vector.tensor_scalar_mul(
                            mov[:, C:NC2], wts, sc[:, J2 + g : J2 + g + 1]
                        )
                    st, sp = (g == 0), (g == J - 1)
                    for m in range(2):
                        nc.tensor.matmul(
                            e_ps[m][:],
                            emb[:, m * 128 : (m + 1) * 128],
                            mov[:],
                            start=st,
                            stop=sp,
                        )

            # ---- phase 2: bias, attn logits, sigmoid, gate, W_out ----
            # eb0 on ACT, eb1 on DVE so they run concurrently
            eb = [
                postp.tile([128, NC2], BF16, name=f"eb{m}", tag=f"eb{m}")
                for m in range(2)
            ]
            nc.scalar.activation(
                eb[0][:],
                e_ps[0][:],
                mybir.ActivationFunctionType.Identity,
                bias=be[:, 0:1],
                scale=1.0,
            )
            nc.vector.tensor_scalar_add(eb[1][:], e_ps[1][:], be[:, 1:2])

            a_ps = [
                psp.tile([128, NC2], F32, name=f"a_ps{jm}", tag=f"a_ps{jm}")
                for jm in range(2)
            ]
            for kt in range(2):
                for jm in range(2):
                    nc.tensor.matmul(
                        a_ps[jm][:],
                        wat[:, kt * K + jm * 128 : kt * K + (jm + 1) * 128],
                        eb[kt][:],
                        start=(kt == 0),
                        stop=(kt == 1),
                    )

            v = []
            for jm in range(2):
                atn = postp.tile([128, NC2], F32, tag=f"atn{jm}")
                nc.scalar.activation(
                    atn[:],
                    a_ps[jm][:],
                    mybir.ActivationFunctionType.Sigmoid,
                    bias=ba[:, jm : jm + 1],
                    scale=1.0,
                )
                v_jm = postp.tile([128, NC2], BF16, tag=f"v{jm}")
                nc.vector.tensor_mul(v_jm[:], atn[:], eb[jm][:])
                v.append(v_jm)

            s_ps = psp.tile([1, NC2], F32, tag="s_ps")
            for kt in range(2):
                nc.tensor.matmul(
                    s_ps[:],
                    wout[:, kt : kt + 1],
                    v[kt][:],
                    start=(kt == 0),
                    stop=(kt == 1),
                )
            s_sb = postp.tile([1, NC2], F32, tag="s_sb")
            nc.scalar.activation(
                s_sb[:],
                s_ps[:],
                mybir.ActivationFunctionType.Identity,
                bias=bo[:],
                scale=1.0,
            )
            nc.sync.dma_start(s_out[:], s_sb[:])

    nc.compile()
    return nc


def _get_nc():
    if "nc" not in _NC_CACHE:
        _NC_CACHE["nc"] = _build_nc()
    return _NC_CACHE["nc"]


def prep_inputs(notevec, wikivec, W_emb, b_emb, W_att, b_att, W_out, b_out):
    # blk[g] = [128, 512] bf16: cols 0:256 wikiT rows g*128..g*128+128,
    # cols 256:512 wembT same rows (zero-padded past V)
    both = np.zeros((J * 128, TW), np.float32)
    both[:V, 0:C] = np.asarray(wikivec, np.float32).T
    both[:V, C:TW] = np.asarray(W_emb, np.float32).T
    # partition-major: blk[p, g*TW + c] = tile g, sbuf partition p, col c
    blk = np.ascontiguousarray(
        both.reshape(J, 128, TW).transpose(1, 0, 2).reshape(128, J * TW)
    ).astype(BF16_NP)

    # watT[p, kt*K + j] = W_att[j, kt*128+p]; cols 512:514 hold W_out
    watT = np.zeros((128, 2 * K + 2), np.float32)
    wa = np.asarray(W_att, np.float32)
    for kt in range(2):
        watT[:, kt * K : (kt + 1) * K] = wa[:, kt * 128 : (kt + 1) * 128].T
    watT[:, 2 * K : 2 * K + 2] = np.asarray(W_out, np.float32)[0].reshape(2, 128).T
    watT = np.ascontiguousarray(watT).astype(BF16_NP)

    nv = np.zeros((N, J2 * 128), np.float32)
    nv[:, :V] = np.asarray(notevec, np.float32)

    pack_common = np.zeros((128, PACK_W), np.float32)
    pack_common[:, 162:164] = np.asarray(b_emb, np.float32).reshape(2, 128).T
    pack_common[:, 164:166] = np.asarray(b_att, np.float32).reshape(2, 128).T
    pack_common[:, 166] = np.asarray(b_out, np.float32)[0]

    in_maps = []
    for i in range(N_CORES):
        pack = pack_common.copy()
        # scales[p, l*J2 + g] = notevec[2i+l, g*128+p]
        pack[:, 0 : NLOC * J2] = (
            nv[i * NLOC : (i + 1) * NLOC]
            .reshape(NLOC, J2, 128)
            .transpose(2, 0, 1)
            .reshape(128, NLOC * J2)
        )
        in_maps.append(
            {
                "blk": blk,
                "pack": np.ascontiguousarray(pack),
                "watT": watT,
            }
        )
    return in_maps


def run(in_maps, **kw):
    nc = _get_nc()
    return run_bass_kernel_spmd(nc, in_maps, list(range(N_CORES)), **kw)


def kernel(notevec, wikivec, W_emb, b_emb, W_att, b_att, W_out, b_out):
    in_maps = prep_inputs(
        notevec, wikivec, W_emb, b_emb, W_att, b_att, W_out, b_out
    )
    res = run(in_maps)
    out = np.concatenate(
        [r["s_out"].reshape(NLOC, C) for r in res.results], axis=0
    )
    return out.astype(np.float32)
